# revision 11
# baseline (speedup 1.0000x reference)
"""GroupQueryAttention TRN2 Bass kernel (fused-pipeline, all-bf16).

Problem: B=4, T=2048, C=1024, H=16 heads, G=4 groups, head_dim=64, causal.
Sharding: 8 cores = 4 batches (DP) x 2 tensor-parallel halves (8 heads /
2 groups each). Host pre-transposes x and weight slices to bf16; each core
computes a partial output projection over its 512 attention channels; host
sums the two TP partials per batch and adds the bias.

v2 changes vs baseline (323us):
  - everything bf16 (proj + attention + out-proj); host converts inputs.
    Halves DMA and lowers PE power. Verified rel err ~2.5e-3 in numpy.
  - single fused pipeline: per tq-block j, attention(j) overlaps with
    projections(j+1) and out-proj(j-1) via interleaved PE instruction
    emission; ACT runs only the exps; Pool (gpsimd) takes psum->sbuf
    copies + diag masks; DVE does v-transposes, normalize, y copies.
  - x DMA'd in j-striped pieces so k/v/q(0) start ~10us earlier.
  - per-head score/PV software pipeline (score(t+1) issued before PV(t))
    so the PE does not stall on the exp latency.
"""

import sys
import numpy as np
import ml_dtypes

for _p in ("/opt/trn_rl_repo", "/opt/trn_rl_repo/concourse"):
    if _p not in sys.path:
        sys.path.insert(0, _p)

import concourse.bass as bass  # noqa: E402
import concourse.mybir as mybir  # noqa: E402
from concourse import bacc  # noqa: E402
from concourse.tile import TileContext  # noqa: E402
from concourse.bass_utils import run_bass_kernel_spmd  # noqa: E402
from concourse.masks import make_identity, make_upper_triangular  # noqa: E402

F32 = mybir.dt.float32
BF16 = mybir.dt.bfloat16
BF = ml_dtypes.bfloat16

B, T, C = 4, 2048, 1024
NH, NG, HD = 16, 4, 64
NH_LOC, NG_LOC = 8, 2          # per-core heads / groups
S = NH_LOC * HD                # 512 local attention channels
TQB = 512                      # tq block
NTQB = T // TQB                # 4
NKT = T // 128                 # 16 tk tiles
NCT = C // 128                 # 8 contraction tiles
SCALE = float(HD) ** -0.5
EXP = mybir.ActivationFunctionType.Exp


def _build_program():
    nc = bacc.Bacc("TRN2", target_bir_lowering=False, debug=False, num_devices=8)

    xT = nc.dram_tensor("xT", [C, T], BF16, kind="ExternalInput")
    wqT = nc.dram_tensor("wqT", [C, S], BF16, kind="ExternalInput")
    wkT = nc.dram_tensor("wkT", [C, NG_LOC * HD], BF16, kind="ExternalInput")
    wvT = nc.dram_tensor("wvT", [C, NG_LOC * HD], BF16, kind="ExternalInput")
    wpT = nc.dram_tensor("wpT", [S, C], BF16, kind="ExternalInput")
    y = nc.dram_tensor("y", [T, C], F32, kind="ExternalOutput")

    with TileContext(nc) as tc:
        with tc.tile_pool(name="const", bufs=1) as const_pool, \
             tc.tile_pool(name="persist", bufs=1) as persist, \
             tc.tile_pool(name="vtp", bufs=2) as vtp, \
             tc.tile_pool(name="pp", bufs=6) as ppool, \
             tc.tile_pool(name="attn", bufs=2) as apool, \
             tc.tile_pool(name="sm", bufs=4) as small, \
             tc.tile_pool(name="yo", bufs=4) as ypool, \
             tc.tile_pool(name="psProj", bufs=2, space="PSUM") as psProj, \
             tc.tile_pool(name="psS", bufs=2, space="PSUM") as psS, \
             tc.tile_pool(name="psO", bufs=2, space="PSUM") as psO, \
             tc.tile_pool(name="psY", bufs=2, space="PSUM") as psY:

            # ---- constants ----
            ident = const_pool.tile([128, 64], F32)
            make_identity(nc, ident[0:64, 0:64])
            make_identity(nc, ident[64:128, 0:64], nomemset=False)
            mask32 = const_pool.tile([128, 128], F32)
            make_upper_triangular(nc, mask32, val=1.0, diag=True)
            mask = const_pool.tile([128, 128], BF16)
            nc.vector.tensor_copy(mask, mask32)

            # ---- persistent SBUF ----
            xA = [persist.tile([128, TQB], BF16, tag=f"xa{ct}", name=f"xa{ct}")
                  for ct in range(NCT)]
            xB = [persist.tile([128, T - TQB], BF16, tag=f"xb{ct}", name=f"xb{ct}")
                  for ct in range(NCT)]
            wq_sb = [persist.tile([128, S], BF16, tag=f"wq{ct}", name=f"wq{ct}")
                     for ct in range(NCT)]
            wk_sb = [persist.tile([128, NG_LOC * HD], BF16, tag=f"wk{ct}", name=f"wk{ct}")
                     for ct in range(NCT)]
            wv_sb = [persist.tile([128, NG_LOC * HD], BF16, tag=f"wv{ct}", name=f"wv{ct}")
                     for ct in range(NCT)]
            wp_sb = [persist.tile([128, C], BF16, tag=f"wp{i}", name=f"wp{i}")
                     for i in range(4)]
            qt_sb = [persist.tile([128, T], BF16, tag=f"qt{i}", name=f"qt{i}")
                     for i in range(4)]
            kdup = [persist.tile([128, T], BF16, tag=f"kd{g}", name=f"kd{g}")
                    for g in range(NG_LOC)]
            # v (transposed): per group [T-on-partitions x 128-tiles]; first 64
            # lhsT cols of each tile are ones so PV emits the softmax
            # denominator rows at zero extra PE cost.
            v_sb = [persist.tile([128, NKT * 128], BF16, tag=f"v{g}", name=f"v{g}")
                    for g in range(NG_LOC)]
            for g in range(NG_LOC):
                nc.vector.memset(v_sb[g], 1.0)

            # ---- input DMAs (order = arrival priority) ----
            for ct in range(NCT):
                rows = slice(ct * 128, (ct + 1) * 128)
                nc.sync.dma_start(out=wk_sb[ct], in_=wkT[rows, :])
                nc.sync.dma_start(out=wv_sb[ct], in_=wvT[rows, :])
            for ct in range(NCT):
                rows = slice(ct * 128, (ct + 1) * 128)
                nc.sync.dma_start(out=xA[ct], in_=xT[rows, 0:TQB])
            for ct in range(NCT):
                rows = slice(ct * 128, (ct + 1) * 128)
                nc.sync.dma_start(out=wq_sb[ct], in_=wqT[rows, :])
            for ct in range(NCT):
                rows = slice(ct * 128, (ct + 1) * 128)
                nc.sync.dma_start(out=xB[ct], in_=xT[rows, TQB:T])
            for i in range(4):
                nc.sync.dma_start(out=wp_sb[i], in_=wpT[i * 128:(i + 1) * 128, :])

            def xcol(ct, j):
                # x columns for tq block j
                if j == 0:
                    return xA[ct]
                return xB[ct][:, (j - 1) * TQB:j * TQB]

            at_tiles = {}

            # ---------------- emission units ----------------
            def emit_kv(j):
                cols = slice(j * TQB, (j + 1) * TQB)
                psk = psProj.tile([128, TQB], F32, tag="pj")
                for ct in range(NCT):
                    nc.tensor.matmul(psk, wk_sb[ct], xcol(ct, j),
                                     start=(ct == 0), stop=(ct == NCT - 1))
                nc.scalar.copy(kdup[0][0:64, cols], psk[0:64, :])
                nc.scalar.copy(kdup[1][64:128, cols], psk[64:128, :])
                # duplicate each group's k onto the other partition half
                nc.sync.dma_start(out=kdup[0][64:128, cols], in_=kdup[0][0:64, cols])
                nc.sync.dma_start(out=kdup[1][0:64, cols], in_=kdup[1][64:128, cols])
                psv = psProj.tile([128, TQB], F32, tag="pj")
                for ct in range(NCT):
                    nc.tensor.matmul(psv, wv_sb[ct], xcol(ct, j),
                                     start=(ct == 0), stop=(ct == NCT - 1))
                vt = vtp.tile([128, TQB], F32, tag="vt")
                nc.vector.tensor_copy(vt, psv)
                return vt

            def emit_trans(j, vt):
                # transpose v [64, T-block] -> [T-block, 64] tiles via PE
                # (one psum tile per transpose: two transposes into the same
                # psum bank error out on hardware)
                for t4 in range(4):
                    t = j * 4 + t4
                    for g in range(NG_LOC):
                        pst = psProj.tile([128, TQB], F32, tag="pj")
                        nc.tensor.transpose(
                            pst[:, 0:64],
                            vt[g * 64:(g + 1) * 64, t4 * 128:(t4 + 1) * 128],
                            ident[g * 64:(g + 1) * 64, 0:64])
                        nc.vector.tensor_copy(
                            v_sb[g][:, t * 128 + 64:t * 128 + 128],
                            pst[:, 0:64])

            def emit_q(j, p4):
                cols = slice(j * TQB, (j + 1) * TQB)
                ps = psProj.tile([128, TQB], F32, tag="pj")
                for ct in range(NCT):
                    nc.tensor.matmul(ps, wq_sb[ct][:, p4 * 128:(p4 + 1) * 128],
                                     xcol(ct, j), start=(ct == 0), stop=(ct == NCT - 1))
                nc.scalar.copy(qt_sb[p4][:, cols], ps)

            def emit_head(j, h):
                g, p4, r = h // 4, h // 2, h % 2
                tq0 = j * TQB
                ntk = 4 * (j + 1)
                kT_g = kdup[g][r * 64:(r + 1) * 64, :]
                qT_h = qt_sb[p4][r * 64:(r + 1) * 64, :]
                po = psO.tile([128, TQB], F32, tag="po")
                pending = None
                for t in range(ntk):
                    c = t - 4 * j
                    off = max(0, c * 128)
                    ps = psS.tile([128, TQB], F32, tag="ps")
                    nc.tensor.matmul(
                        ps[:, off:TQB],
                        kT_g[:, t * 128:(t + 1) * 128],
                        qT_h[:, tq0 + off:tq0 + TQB],
                        start=True, stop=True)
                    pt = ppool.tile([128, TQB], BF16, tag="pt")
                    nc.scalar.activation(pt[:, off:TQB], ps[:, off:TQB],
                                         EXP, scale=SCALE)
                    if c >= 0:
                        nc.vector.tensor_mul(
                            pt[:, off:off + 128], pt[:, off:off + 128], mask)
                    if pending is not None:
                        t_, pt_, off_ = pending
                        nc.tensor.matmul(
                            po[:, off_:TQB],
                            v_sb[g][:, t_ * 128:(t_ + 1) * 128],
                            pt_[:, off_:TQB],
                            start=(t_ == 0), stop=False)
                    pending = (t, pt, off)
                t_, pt_, off_ = pending
                nc.tensor.matmul(
                    po[:, off_:TQB],
                    v_sb[g][:, t_ * 128:(t_ + 1) * 128],
                    pt_[:, off_:TQB],
                    start=(t_ == 0), stop=True)
                # normalize: out = po[64:128] / po[0:64]
                rcp = small.tile([128, TQB], F32, tag="recip")
                nc.vector.reciprocal_approx_fast(rcp[0:64, :], po[0:64, :])
                nc.vector.tensor_mul(
                    at_tiles[j][p4][r * 64:(r + 1) * 64, :],
                    po[64:128, :], rcp[0:64, :])

            def emit_yproj(j, tt):
                tau = j * 4 + tt
                ysb = ypool.tile([128, C], F32, tag="y")
                for half in range(2):
                    yp = psY.tile([128, TQB], F32, tag="yp")
                    for p4 in range(4):
                        nc.tensor.matmul(
                            yp,
                            at_tiles[j][p4][:, tt * 128:(tt + 1) * 128],
                            wp_sb[p4][:, half * TQB:(half + 1) * TQB],
                            start=(p4 == 0), stop=(p4 == 3))
                    nc.vector.tensor_copy(ysb[:, half * TQB:(half + 1) * TQB], yp)
                nc.sync.dma_start(out=y[tau * 128:(tau + 1) * 128, :], in_=ysb)

            def emit_kv_trans(j):
                vt = emit_kv(j)
                emit_trans(j, vt)

            def emit_proj(j):
                emit_kv_trans(j)
                for p4 in range(4):
                    emit_q(j, p4)

            # ---------------- schedule ----------------
            INTERLEAVE = True
            emit_proj(0)

            # head order: dup-free heads first (g0/r0, g1/r1), then the ones
            # needing the kdup duplication DMA
            HEAD_ORDER = [0, 5, 2, 7, 4, 1, 6, 3]

            if INTERLEAVE:
                for j in range(NTQB):
                    at_tiles[j] = [apool.tile([128, TQB], BF16, tag=f"at{p4}",
                                              name=f"at{j}_{p4}")
                                   for p4 in range(4)]
                    fillers = []
                    if j + 1 < NTQB:
                        fillers.append(lambda jj=j + 1: emit_kv_trans(jj))
                        for p4 in range(4):
                            fillers.append(lambda jj=j + 1, pp4=p4: emit_q(jj, pp4))
                    if j >= 1:
                        for tt in range(4):
                            fillers.append(lambda jj=j - 1, tt_=tt: emit_yproj(jj, tt_))
                    done = 0
                    for i, h in enumerate(HEAD_ORDER):
                        emit_head(j, h)
                        want = (i + 1) * len(fillers) // len(HEAD_ORDER)
                        while done < want:
                            fillers[done]()
                            done += 1
                for tt in range(4):
                    emit_yproj(NTQB - 1, tt)
            else:
                for j in range(NTQB):
                    at_tiles[j] = [apool.tile([128, TQB], BF16, tag=f"at{p4}",
                                              name=f"at{j}_{p4}")
                                   for p4 in range(4)]
                    if j > 0:
                        emit_proj(j)
                    for h in HEAD_ORDER:
                        emit_head(j, h)
                    for tt in range(4):
                        emit_yproj(j, tt)

    nc.compile()
    return nc


_NC_CACHE = None


def _get_nc():
    global _NC_CACHE
    if _NC_CACHE is None:
        _NC_CACHE = _build_program()
    return _NC_CACHE


def _make_in_maps(x, Wq, Wk, Wv, Wp):
    in_maps = []
    for core in range(8):
        b, tp = core // 2, core % 2
        hs = slice(tp * NH_LOC, (tp + 1) * NH_LOC)
        gs = slice(tp * NG_LOC, (tp + 1) * NG_LOC)
        in_maps.append({
            "xT": np.ascontiguousarray(x[b].T).astype(BF),
            "wqT": np.ascontiguousarray(
                Wq[hs].transpose(2, 0, 1).reshape(C, S)).astype(BF),
            "wkT": np.ascontiguousarray(
                Wk[gs].transpose(2, 0, 1).reshape(C, NG_LOC * HD)).astype(BF),
            "wvT": np.ascontiguousarray(
                Wv[gs].transpose(2, 0, 1).reshape(C, NG_LOC * HD)).astype(BF),
            "wpT": np.ascontiguousarray(Wp[:, tp * S:(tp + 1) * S].T).astype(BF),
        })
    return in_maps


def kernel(x, Wq, Wk, Wv, Wp, bp, _trace=False):
    x = np.asarray(x, dtype=np.float32)
    nc = _get_nc()
    in_maps = _make_in_maps(
        x, np.asarray(Wq, np.float32), np.asarray(Wk, np.float32),
        np.asarray(Wv, np.float32), np.asarray(Wp, np.float32))
    res = run_bass_kernel_spmd(nc, in_maps, list(range(8)), trace=_trace)
    out = np.empty((B, T, C), dtype=np.float32)
    bp32 = np.asarray(bp, np.float32)
    for b in range(B):
        out[b] = res.results[2 * b]["y"] + res.results[2 * b + 1]["y"] + bp32
    if _trace:
        return out, res
    return out


# revision 12
# speedup vs baseline: 1.1147x; 1.1147x over previous
"""GroupQueryAttention TRN2 Bass kernel (fused-pipeline, all-bf16).

Problem: B=4, T=2048, C=1024, H=16 heads, G=4 groups, head_dim=64, causal.
Sharding: 8 cores = 4 batches (DP) x 2 tensor-parallel halves (8 heads /
2 groups each). Host pre-transposes x and weight slices to bf16; each core
computes a partial output projection over its 512 attention channels; host
sums the two TP partials per batch and adds the bias.

v2 changes vs baseline (323us):
  - everything bf16 (proj + attention + out-proj); host converts inputs.
    Halves DMA and lowers PE power. Verified rel err ~2.5e-3 in numpy.
  - single fused pipeline: per tq-block j, attention(j) overlaps with
    projections(j+1) and out-proj(j-1) via interleaved PE instruction
    emission; ACT runs only the exps; Pool (gpsimd) takes psum->sbuf
    copies + diag masks; DVE does v-transposes, normalize, y copies.
  - x DMA'd in j-striped pieces so k/v/q(0) start ~10us earlier.
  - per-head score/PV software pipeline (score(t+1) issued before PV(t))
    so the PE does not stall on the exp latency.
"""

import sys
import numpy as np
import ml_dtypes

for _p in ("/opt/trn_rl_repo", "/opt/trn_rl_repo/concourse"):
    if _p not in sys.path:
        sys.path.insert(0, _p)

import concourse.bass as bass  # noqa: E402
import concourse.mybir as mybir  # noqa: E402
from concourse import bacc  # noqa: E402
from concourse.tile import TileContext  # noqa: E402
from concourse.bass_utils import run_bass_kernel_spmd  # noqa: E402
from concourse.masks import make_identity, make_upper_triangular  # noqa: E402

F32 = mybir.dt.float32
F32R = mybir.dt.float32r
BF16 = mybir.dt.bfloat16
BF = ml_dtypes.bfloat16

B, T, C = 4, 2048, 1024
NH, NG, HD = 16, 4, 64
NH_LOC, NG_LOC = 8, 2          # per-core heads / groups
S = NH_LOC * HD                # 512 local attention channels
TQB = 512                      # tq block
NTQB = T // TQB                # 4
NKT = T // 128                 # 16 tk tiles
NCT = C // 128                 # 8 contraction tiles
SCALE = float(HD) ** -0.5
EXP = mybir.ActivationFunctionType.Exp


def _build_program():
    nc = bacc.Bacc("TRN2", target_bir_lowering=False, debug=False, num_devices=8)

    xT = nc.dram_tensor("xT", [C, T], F32R, kind="ExternalInput")
    wqT = nc.dram_tensor("wqT", [C, S], F32R, kind="ExternalInput")
    wkT = nc.dram_tensor("wkT", [C, NG_LOC * HD], F32R, kind="ExternalInput")
    wvT = nc.dram_tensor("wvT", [C, NG_LOC * HD], F32R, kind="ExternalInput")
    wpT = nc.dram_tensor("wpT", [S, C], F32R, kind="ExternalInput")
    y = nc.dram_tensor("y", [T, C], F32, kind="ExternalOutput")

    with TileContext(nc) as tc:
        with tc.tile_pool(name="const", bufs=1) as const_pool, \
             tc.tile_pool(name="persist", bufs=1) as persist, \
             tc.tile_pool(name="vtp", bufs=2) as vtp, \
             tc.tile_pool(name="pp", bufs=6) as ppool, \
             tc.tile_pool(name="attn", bufs=2) as apool, \
             tc.tile_pool(name="sm", bufs=4) as small, \
             tc.tile_pool(name="yo", bufs=4) as ypool, \
             tc.tile_pool(name="psProj", bufs=2, space="PSUM") as psProj, \
             tc.tile_pool(name="psS", bufs=2, space="PSUM") as psS, \
             tc.tile_pool(name="psO", bufs=2, space="PSUM") as psO, \
             tc.tile_pool(name="psY", bufs=2, space="PSUM") as psY:

            # ---- constants ----
            ident = const_pool.tile([128, 64], F32)
            make_identity(nc, ident[0:64, 0:64])
            make_identity(nc, ident[64:128, 0:64], nomemset=False)
            mask32 = const_pool.tile([128, 128], F32)
            make_upper_triangular(nc, mask32, val=1.0, diag=True)
            mask = const_pool.tile([128, 128], BF16)
            nc.vector.tensor_copy(mask, mask32)

            # ---- persistent SBUF ----
            xA = [persist.tile([128, TQB], F32R, tag=f"xa{ct}", name=f"xa{ct}")
                  for ct in range(NCT)]
            xB = [persist.tile([128, T - TQB], F32R, tag=f"xb{ct}", name=f"xb{ct}")
                  for ct in range(NCT)]
            wq_sb = [persist.tile([128, S], F32R, tag=f"wq{ct}", name=f"wq{ct}")
                     for ct in range(NCT)]
            wk_sb = [persist.tile([128, NG_LOC * HD], F32R, tag=f"wk{ct}", name=f"wk{ct}")
                     for ct in range(NCT)]
            wv_sb = [persist.tile([128, NG_LOC * HD], F32R, tag=f"wv{ct}", name=f"wv{ct}")
                     for ct in range(NCT)]
            wp_sb = [persist.tile([128, C], F32R, tag=f"wp{i}", name=f"wp{i}")
                     for i in range(4)]
            qt_sb = [persist.tile([128, T], BF16, tag=f"qt{i}", name=f"qt{i}")
                     for i in range(4)]
            kdup = [persist.tile([128, T], BF16, tag=f"kd{g}", name=f"kd{g}")
                    for g in range(NG_LOC)]
            # v (transposed): per group [T-on-partitions x 128-tiles]; first 64
            # lhsT cols of each tile are ones so PV emits the softmax
            # denominator rows at zero extra PE cost.
            v_sb = [persist.tile([128, NKT * 128], BF16, tag=f"v{g}", name=f"v{g}")
                    for g in range(NG_LOC)]
            for g in range(NG_LOC):
                nc.vector.memset(v_sb[g], 1.0)

            # ---- input DMAs (order = arrival priority) ----
            for ct in range(NCT):
                rows = slice(ct * 128, (ct + 1) * 128)
                nc.sync.dma_start(out=wk_sb[ct], in_=wkT[rows, :])
                nc.sync.dma_start(out=wv_sb[ct], in_=wvT[rows, :])
            for ct in range(NCT):
                rows = slice(ct * 128, (ct + 1) * 128)
                nc.sync.dma_start(out=xA[ct], in_=xT[rows, 0:TQB])
            for ct in range(NCT):
                rows = slice(ct * 128, (ct + 1) * 128)
                nc.sync.dma_start(out=wq_sb[ct], in_=wqT[rows, :])
            for ct in range(NCT):
                rows = slice(ct * 128, (ct + 1) * 128)
                nc.sync.dma_start(out=xB[ct], in_=xT[rows, TQB:T])
            for i in range(4):
                nc.sync.dma_start(out=wp_sb[i], in_=wpT[i * 128:(i + 1) * 128, :])

            def xcol(ct, j):
                # x columns for tq block j
                if j == 0:
                    return xA[ct]
                return xB[ct][:, (j - 1) * TQB:j * TQB]

            at_tiles = {}

            # ---------------- emission units ----------------
            def emit_kv(j):
                cols = slice(j * TQB, (j + 1) * TQB)
                psk = psProj.tile([128, TQB], F32, tag="pj")
                for ct in range(NCT):
                    nc.tensor.matmul(psk, wk_sb[ct], xcol(ct, j),
                                     start=(ct == 0), stop=(ct == NCT - 1))
                nc.scalar.copy(kdup[0][0:64, cols], psk[0:64, :])
                nc.scalar.copy(kdup[1][64:128, cols], psk[64:128, :])
                # duplicate each group's k onto the other partition half
                nc.sync.dma_start(out=kdup[0][64:128, cols], in_=kdup[0][0:64, cols])
                nc.sync.dma_start(out=kdup[1][0:64, cols], in_=kdup[1][64:128, cols])
                psv = psProj.tile([128, TQB], F32, tag="pj")
                for ct in range(NCT):
                    nc.tensor.matmul(psv, wv_sb[ct], xcol(ct, j),
                                     start=(ct == 0), stop=(ct == NCT - 1))
                vt = vtp.tile([128, TQB], F32, tag="vt")
                nc.vector.tensor_copy(vt, psv)
                return vt

            def emit_trans(j, vt):
                # transpose v [64, T-block] -> [T-block, 64] tiles via PE
                # (one psum tile per transpose: two transposes into the same
                # psum bank error out on hardware)
                for t4 in range(4):
                    t = j * 4 + t4
                    for g in range(NG_LOC):
                        pst = psProj.tile([128, TQB], F32, tag="pj")
                        nc.tensor.transpose(
                            pst[:, 0:64],
                            vt[g * 64:(g + 1) * 64, t4 * 128:(t4 + 1) * 128],
                            ident[g * 64:(g + 1) * 64, 0:64])
                        nc.vector.tensor_copy(
                            v_sb[g][:, t * 128 + 64:t * 128 + 128],
                            pst[:, 0:64])

            def emit_q(j, p4):
                cols = slice(j * TQB, (j + 1) * TQB)
                ps = psProj.tile([128, TQB], F32, tag="pj")
                for ct in range(NCT):
                    nc.tensor.matmul(ps, wq_sb[ct][:, p4 * 128:(p4 + 1) * 128],
                                     xcol(ct, j), start=(ct == 0), stop=(ct == NCT - 1))
                nc.scalar.copy(qt_sb[p4][:, cols], ps)

            def emit_head(j, h):
                g, p4, r = h // 4, h // 2, h % 2
                tq0 = j * TQB
                ntk = 4 * (j + 1)
                kT_g = kdup[g][r * 64:(r + 1) * 64, :]
                qT_h = qt_sb[p4][r * 64:(r + 1) * 64, :]
                po = psO.tile([128, TQB], F32, tag="po")
                pending = None
                for t in range(ntk):
                    c = t - 4 * j
                    off = max(0, c * 128)
                    ps = psS.tile([128, TQB], F32, tag="ps")
                    nc.tensor.matmul(
                        ps[:, off:TQB],
                        kT_g[:, t * 128:(t + 1) * 128],
                        qT_h[:, tq0 + off:tq0 + TQB],
                        start=True, stop=True)
                    pt = ppool.tile([128, TQB], BF16, tag="pt")
                    nc.scalar.activation(pt[:, off:TQB], ps[:, off:TQB],
                                         EXP, scale=SCALE)
                    if c >= 0:
                        nc.vector.tensor_mul(
                            pt[:, off:off + 128], pt[:, off:off + 128], mask)
                    if pending is not None:
                        t_, pt_, off_ = pending
                        nc.tensor.matmul(
                            po[:, off_:TQB],
                            v_sb[g][:, t_ * 128:(t_ + 1) * 128],
                            pt_[:, off_:TQB],
                            start=(t_ == 0), stop=False)
                    pending = (t, pt, off)
                t_, pt_, off_ = pending
                nc.tensor.matmul(
                    po[:, off_:TQB],
                    v_sb[g][:, t_ * 128:(t_ + 1) * 128],
                    pt_[:, off_:TQB],
                    start=(t_ == 0), stop=True)
                # normalize: out = po[64:128] / po[0:64]
                rcp = small.tile([128, TQB], F32, tag="recip")
                nc.vector.reciprocal_approx_fast(rcp[0:64, :], po[0:64, :])
                nc.vector.tensor_mul(
                    at_tiles[j][p4][r * 64:(r + 1) * 64, :],
                    po[64:128, :], rcp[0:64, :])

            def emit_yproj(j, tt):
                tau = j * 4 + tt
                ysb = ypool.tile([128, C], F32, tag="y")
                for half in range(2):
                    yp = psY.tile([128, TQB], F32, tag="yp")
                    for p4 in range(4):
                        nc.tensor.matmul(
                            yp,
                            at_tiles[j][p4][:, tt * 128:(tt + 1) * 128],
                            wp_sb[p4][:, half * TQB:(half + 1) * TQB],
                            start=(p4 == 0), stop=(p4 == 3))
                    nc.vector.tensor_copy(ysb[:, half * TQB:(half + 1) * TQB], yp)
                nc.sync.dma_start(out=y[tau * 128:(tau + 1) * 128, :], in_=ysb)

            def emit_kv_trans(j):
                vt = emit_kv(j)
                emit_trans(j, vt)

            def emit_proj(j):
                emit_kv_trans(j)
                for p4 in range(4):
                    emit_q(j, p4)

            # ---------------- schedule ----------------
            INTERLEAVE = True
            emit_proj(0)

            # head order: dup-free heads first (g0/r0, g1/r1), then the ones
            # needing the kdup duplication DMA
            HEAD_ORDER = [0, 5, 2, 7, 4, 1, 6, 3]

            if INTERLEAVE:
                for j in range(NTQB):
                    at_tiles[j] = [apool.tile([128, TQB], F32R, tag=f"at{p4}",
                                              name=f"at{j}_{p4}")
                                   for p4 in range(4)]
                    fillers = []
                    if j + 1 < NTQB:
                        fillers.append(lambda jj=j + 1: emit_kv_trans(jj))
                        for p4 in range(4):
                            fillers.append(lambda jj=j + 1, pp4=p4: emit_q(jj, pp4))
                    if j >= 1:
                        for tt in range(4):
                            fillers.append(lambda jj=j - 1, tt_=tt: emit_yproj(jj, tt_))
                    done = 0
                    for i, h in enumerate(HEAD_ORDER):
                        emit_head(j, h)
                        want = (i + 1) * len(fillers) // len(HEAD_ORDER)
                        while done < want:
                            fillers[done]()
                            done += 1
                for tt in range(4):
                    emit_yproj(NTQB - 1, tt)
            else:
                for j in range(NTQB):
                    at_tiles[j] = [apool.tile([128, TQB], F32R, tag=f"at{p4}",
                                              name=f"at{j}_{p4}")
                                   for p4 in range(4)]
                    if j > 0:
                        emit_proj(j)
                    for h in HEAD_ORDER:
                        emit_head(j, h)
                    for tt in range(4):
                        emit_yproj(j, tt)

    nc.compile()
    return nc


_NC_CACHE = None


def _get_nc():
    global _NC_CACHE
    if _NC_CACHE is None:
        _NC_CACHE = _build_program()
    return _NC_CACHE


def _make_in_maps(x, Wq, Wk, Wv, Wp):
    in_maps = []
    for core in range(8):
        b, tp = core // 2, core % 2
        hs = slice(tp * NH_LOC, (tp + 1) * NH_LOC)
        gs = slice(tp * NG_LOC, (tp + 1) * NG_LOC)
        in_maps.append({
            "xT": np.ascontiguousarray(x[b].T),
            "wqT": np.ascontiguousarray(
                Wq[hs].transpose(2, 0, 1).reshape(C, S)),
            "wkT": np.ascontiguousarray(
                Wk[gs].transpose(2, 0, 1).reshape(C, NG_LOC * HD)),
            "wvT": np.ascontiguousarray(
                Wv[gs].transpose(2, 0, 1).reshape(C, NG_LOC * HD)),
            "wpT": np.ascontiguousarray(Wp[:, tp * S:(tp + 1) * S].T),
        })
    return in_maps


def kernel(x, Wq, Wk, Wv, Wp, bp, _trace=False):
    x = np.asarray(x, dtype=np.float32)
    nc = _get_nc()
    in_maps = _make_in_maps(
        x, np.asarray(Wq, np.float32), np.asarray(Wk, np.float32),
        np.asarray(Wv, np.float32), np.asarray(Wp, np.float32))
    res = run_bass_kernel_spmd(nc, in_maps, list(range(8)), trace=_trace)
    out = np.empty((B, T, C), dtype=np.float32)
    bp32 = np.asarray(bp, np.float32)
    for b in range(B):
        out[b] = res.results[2 * b]["y"] + res.results[2 * b + 1]["y"] + bp32
    if _trace:
        return out, res
    return out
